# revision 15
# baseline (speedup 1.0000x reference)
"""Trainium2 Bass kernel for nn_CrossAttention (single-head NxN attention + proj + InstanceNorm + residual).

Sharding: 8 cores = (batch b in 0..3) x (query-half h in 0..1).
Each core computes its half of the query tokens for one batch; the
InstanceNorm statistics (over the full 4096 tokens) are combined across
the core pair with a tiny AllGather (a dummy warmup AllGather at kernel
start absorbs the ~45us first-collective cost).

v3 structure:
 - PE clock warmup: dummy matmuls during the input-DMA prologue keep the
   PE HAM activity monitor busy so the first real matmuls run at 2.4 GHz.
 - Coarse input DMA: weights are host-concatenated into one tensor and
   the whole input set moves in 8 large descriptors (descriptor issue
   costs ~0.6us each and rings only keep ~2 in flight).
 - i-tiles of 512 queries: QK runs as 4 concurrent 32-row-group matmuls
   (one per j-block) with 512-wide moving operands -- ~3x fewer PE
   cycles than 256-wide 2-way-concurrent tiles.  One exp() activation
   per superburst covers [128, 2048] (less ACT overhead).
 - PSUM: qk [128,4,512] (4 banks) + 3 av accumulator banks holding 8
   packed regions (per 128-query chunk: [c0..127+denom] and [c128..255])
   + 1 bank for transpose/proj/v-prologue tiles.
 - Raw per-i-tile bn_stats are exchanged in the AllGather, staged into
   the collective input DRAM as produced; the pair combination is two
   bn_aggr calls.

Precision: fp16 matmul operands everywhere, fp32 PSUM accumulation; the
qk*Cr^-0.5 scale is folded into the exp() activation scale.

Self-contained: hardcodes shapes B=4, C=256, D=H=W=16 (N=4096), Cr=32.
"""

import numpy as np

import concourse.bass as bass
import concourse.mybir as mybir
import concourse.tile as tile
from concourse import bacc
from concourse.bass_utils import run_bass_kernel_spmd
from concourse.masks import make_identity

B, C, N, Cr = 4, 256, 4096, 32
NH = N // 2  # query tokens per core
EPS = 1e-5
SCALE = float(Cr) ** -0.5
FP32 = mybir.dt.float32
FP16 = mybir.dt.float16

N_CORES = 8
REPLICA_GROUPS = [[0, 1], [2, 3], [4, 5], [6, 7]]

IT = 512                   # i-tile width (query columns per superburst)
N_ITILES = NH // IT        # 4
JBLK = 128                 # j-block (rows per QK matmul output)
N_JBLK = N // JBLK         # 32
JB_PER_SB = 4              # j-blocks per superburst (4-way row-tiled QK)
SB_PER_IT = N_JBLK // JB_PER_SB  # 8
N_SB = N_ITILES * SB_PER_IT      # 32

# av accumulator packing: 8 regions (4 query-chunks x {A: c0-127+denom,
# B: c128-255}) packed 3 per PSUM bank at 136-col stride
AV_REGION_STRIDE = 136

N_WARM_MM = 8              # dummy matmuls to warm the PE clock gate

AF = mybir.ActivationFunctionType
ALU = mybir.AluOpType

LAST_RESULTS = None  # BassKernelResults of the most recent run (for test harness)


def build_nc(use_collective=True):
    nc = bacc.Bacc("TRN2", num_devices=N_CORES, name="xattn",
                   target_bir_lowering=False)

    x1h_d = nc.dram_tensor("x1h", [C, NH], FP16, kind="ExternalInput").ap()
    x2b_d = nc.dram_tensor("x2b", [C, N], FP16, kind="ExternalInput").ap()
    # wv[256] | wk[128] | wq[128] | wp[256] concatenated along the free dim
    wcat_d = nc.dram_tensor("wcat", [C, 768], FP16, kind="ExternalInput").ap()
    out_d = nc.dram_tensor("out", [C, NH], FP32, kind="ExternalOutput").ap()

    with tile.TileContext(nc) as tc:
        build_body(tc, x1h_d, x2b_d, wcat_d, out_d, use_collective)
    nc.compile()
    return nc


def build_body(tc, x1h_d, x2b_d, wcat_d, out_d, use_collective=True):
    nc = tc.nc
    from contextlib import ExitStack

    with ExitStack() as ctx:
        persist = ctx.enter_context(tc.tile_pool(name="persist", bufs=1))
        sm = ctx.enter_context(tc.tile_pool(name="sm", bufs=4))
        avcp = ctx.enter_context(tc.tile_pool(name="avcp", bufs=2))
        ptp = ctx.enter_context(tc.tile_pool(name="ptp", bufs=3))
        qkp = ctx.enter_context(tc.tile_pool(name="qkp", bufs=1, space="PSUM"))
        avp = ctx.enter_context(tc.tile_pool(name="avp", bufs=3, space="PSUM"))
        # one bank shared by the prologue vp tiles and the epilogue tp/pj
        # tiles (PSUM pool slots are bank-rounded, so bufs=1)
        epi = ctx.enter_context(tc.tile_pool(name="epi", bufs=1, space="PSUM"))
        dramp = ctx.enter_context(tc.tile_pool(name="dramp", bufs=1, space="DRAM"))

        # ---- PE clock warmup: dummy matmuls on a memset tile ------------
        warm_mm = persist.tile([128, 512], FP16, tag="warm_mm", name="warm_mm")
        nc.vector.memset(warm_mm, 0.0)
        warm_ps = qkp.tile([128, 512], FP32, tag="qk", name="warm_ps")
        for w in range(N_WARM_MM):
            nc.tensor.matmul(warm_ps, lhsT=warm_mm[:, 0:128], rhs=warm_mm,
                             start=True, stop=True)

        # ---- warmup collective ------------------------------------------
        if use_collective:
            warm_sb = persist.tile([128, 4], FP32, tag="warm", name="warm_sb")
            nc.vector.memset(warm_sb, 0.0)
            warm_in = dramp.tile([128, 4], FP32, tag="warm_i", name="warm_in")
            warm_out = dramp.tile([2, 128, 4], FP32, tag="warm_o", name="warm_out")
            nc.sync.dma_start(warm_in, warm_sb)
            nc.gpsimd.collective_compute(
                "AllGather", ALU.bypass, replica_groups=REPLICA_GROUPS,
                ins=[warm_in.opt()], outs=[warm_out.opt()])

        # ---- constants -------------------------------------------------
        eps_sb = persist.tile([128, 1], FP32, tag="eps", name="eps_sb")
        nc.vector.memset(eps_sb, EPS)
        ident = persist.tile([128, 128], FP16, tag="ident", name="ident")
        make_identity(nc, ident)

        # ---- input DMAs (large descriptors, weights first) -------------
        wcat_sb = [persist.tile([128, 768], FP16, tag=f"wcat{cc}",
                                name=f"wcat_sb{cc}") for cc in range(2)]
        wv_sb = [wcat_sb[cc][:, 0:256] for cc in range(2)]
        wk_sb = [wcat_sb[cc][:, 256:384] for cc in range(2)]
        wq_sb = [wcat_sb[cc][:, 384:512] for cc in range(2)]
        wp_sb = [wcat_sb[cc][:, 512:768] for cc in range(2)]
        x2_sb = [persist.tile([128, N], FP16, tag=f"x2_{cc}", name=f"x2_sb{cc}")
                 for cc in range(2)]
        x1_sb = [persist.tile([128, NH], FP16, tag=f"x1_{cc}", name=f"x1_sb{cc}")
                 for cc in range(2)]
        for cc in range(2):
            sl = slice(128 * cc, 128 * (cc + 1))
            nc.scalar.dma_start(wcat_sb[cc], wcat_d[sl, :])
        x2q = [nc.sync, nc.gpsimd]
        for ch in range(2):
            slh = slice(2048 * ch, 2048 * (ch + 1))
            for cc in range(2):
                x2q[cc].dma_start(x2_sb[cc][:, slh],
                                  x2b_d[128 * cc:128 * (cc + 1), slh])
        for cc in range(2):
            nc.scalar.dma_start(x1_sb[cc], x1h_d[128 * cc:128 * (cc + 1), :])

        # ---- prologue: vt / k_rep / q_rep ------------------------------
        # vt[j, :] = [v(c0..127) | ones | v(c128..255)] per j-block; the
        # ones column rides the A-half AV matmul as the softmax denominator
        vt = persist.tile([128, N_JBLK, C + 1], FP16, tag="vt", name="vt")
        nc.vector.memset(vt[:, :, 128:129], 1.0)
        k_rep = persist.tile([128, N], FP16, tag="krep", name="k_rep")
        q_rep = persist.tile([128, NH], FP16, tag="qrep", name="q_rep")

        def emit_v(jblk):
            vp = epi.tile([128, C], FP32, tag="epi", name=f"vp{jblk}")
            for cc in range(2):
                nc.tensor.matmul(
                    vp, lhsT=x2_sb[cc][:, 128 * jblk:128 * (jblk + 1)],
                    rhs=wv_sb[cc], start=(cc == 0), stop=(cc == 1))
            if jblk % 2 == 0:
                nc.vector.tensor_copy(vt[:, jblk, 0:128], vp[:, 0:128])
                nc.scalar.copy(vt[:, jblk, 129:257], vp[:, 128:256])
            else:
                nc.scalar.copy(vt[:, jblk, 0:128], vp[:, 0:128])
                nc.vector.tensor_copy(vt[:, jblk, 129:257], vp[:, 128:256])

        def emit_kq(which, u):
            w_sb, dst = (wk_sb, k_rep) if which == "k" else (wq_sb, q_rep)
            src = x2_sb if which == "k" else x1_sb
            kp = avp.tile([128, 512], FP32, tag="av", name=f"{which}p{u}")
            for cc in range(2):
                nc.tensor.matmul(
                    kp, lhsT=w_sb[cc],
                    rhs=src[cc][:, 512 * u:512 * (u + 1)],
                    start=(cc == 0), stop=(cc == 1))
            nc.vector.tensor_copy(dst[:, 512 * u:512 * (u + 1)], kp)

        # k/q prologue (x2 chunk 0 covers k0-3, chunk 1 covers k4-7)
        for u in range(4):
            emit_kq("k", u)
            emit_kq("q", u)
        for u in range(4, 8):
            emit_kq("k", u)

        # ---- persistent attention outputs ------------------------------
        proj_sb = [persist.tile([128, NH], FP32, tag=f"proj{ob}",
                                name=f"proj_sb{ob}") for ob in range(2)]
        stats_sb = [persist.tile([128, N_ITILES, 6], FP32, tag=f"stats{ob}",
                                 name=f"stats_sb{ob}") for ob in range(2)]
        if use_collective:
            ccin_dr = dramp.tile([128, N_ITILES, 2, 6], FP32, tag="ccin_d",
                                 name="ccin_dr")
            ccout_dr = dramp.tile([2, 128, N_ITILES, 2, 6], FP32,
                                  tag="ccout_d", name="ccout_dr")

        # ---- main loop: 32 superbursts of (4-way QK, one exp, AV) ------
        def emit_qk(s):
            it, sbj = s // SB_PER_IT, s % SB_PER_IT
            isl = slice(IT * it, IT * (it + 1))
            qk = qkp.tile([128, JB_PER_SB, IT], FP32, tag="qk", name=f"qk{s}")
            for g in range(JB_PER_SB):
                jblk = sbj * JB_PER_SB + g
                nc.tensor.matmul(
                    qk[:, g, :],
                    lhsT=k_rep[32 * g:32 * (g + 1),
                               JBLK * jblk:JBLK * (jblk + 1)],
                    rhs=q_rep[32 * g:32 * (g + 1), isl],
                    start=True, stop=True, tile_position=(32 * g, 0))
            pt = ptp.tile([128, JB_PER_SB, IT], FP16, tag="pt", name=f"pt{s}")
            nc.scalar.activation(out=pt, in_=qk, func=AF.Exp, scale=SCALE)
            return pt

        def av_region(av_t, rid):
            bank, slot = rid // 3, rid % 3
            w = 129 if rid % 2 == 0 else 128
            off = AV_REGION_STRIDE * slot
            return av_t[bank][:, off:off + w]

        def emit_av(s, pt, av_t):
            it, sbj = s // SB_PER_IT, s % SB_PER_IT
            first = sbj == 0
            last = sbj == SB_PER_IT - 1
            for g in range(JB_PER_SB):
                jblk = sbj * JB_PER_SB + g
                for ic in range(IT // 128):
                    lhsT = pt[:, g, 128 * ic:128 * (ic + 1)]
                    for ab in range(2):
                        rid = ic * 2 + ab
                        rhs = (vt[:, jblk, 0:129] if ab == 0
                               else vt[:, jblk, 129:257])
                        nc.tensor.matmul(
                            av_region(av_t, rid), lhsT=lhsT, rhs=rhs,
                            # start=True clears the has_written bits of the
                            # whole bank: only the first region per bank may
                            # set it (rids 0,3,6 are first in banks 0,1,2);
                            # the other regions' first writes overwrite via
                            # unset bits.
                            start=(first and g == 0 and rid in (0, 3, 6)),
                            stop=(last and g == JB_PER_SB - 1))

        avc = {}

        def epi_chunk_a(it, ic, av_t):
            a = av_region(av_t, ic * 2)
            b = av_region(av_t, ic * 2 + 1)
            rden = sm.tile([128, 1], FP32, tag="rden", name=f"rden{it}_{ic}")
            nc.vector.reciprocal(rden, a[:, 128:129])
            avn = sm.tile([128, C], FP16, tag="avn", name=f"avn{it}_{ic}")
            nc.vector.tensor_scalar_mul(avn[:, 0:128], in0=a[:, 0:128],
                                        scalar1=rden)
            nc.vector.tensor_scalar_mul(avn[:, 128:256], in0=b, scalar1=rden)
            tp = epi.tile([128, C], FP16, tag="epi", name=f"tp{it}_{ic}")
            nc.tensor.transpose(tp[:, 0:128], avn[:, 0:128], ident)
            nc.tensor.transpose(tp[:, 128:256], avn[:, 128:256], ident)
            for cc in range(2):
                nc.vector.tensor_copy(
                    avc[(it, cc)][:, 128 * ic:128 * (ic + 1)],
                    tp[:, 128 * cc:128 * (cc + 1)])

        def epi_chunk_b(it, ob, ih):
            isl = slice(IT * it + 256 * ih, IT * it + 256 * (ih + 1))
            pj = epi.tile([128, 256], FP32, tag="epi", name=f"pj{it}_{ob}_{ih}")
            for cc in range(2):
                nc.tensor.matmul(
                    pj, lhsT=wp_sb[cc][:, 128 * ob:128 * (ob + 1)],
                    rhs=avc[(it, cc)][:, 256 * ih:256 * (ih + 1)],
                    start=(cc == 0), stop=(cc == 1))
            nc.vector.tensor_copy(proj_sb[ob][:, isl], pj)
            if ih == 1:
                tsl = slice(IT * it, IT * (it + 1))
                nc.vector.bn_stats(stats_sb[ob][:, it, :], proj_sb[ob][:, tsl])
                if use_collective:
                    nc.sync.dma_start(ccin_dr[:, it, ob],
                                      stats_sb[ob][:, it, :])

        pending = []
        pts = {0: emit_qk(0)}
        av_t = None
        for s in range(N_SB):
            it, sbj = s // SB_PER_IT, s % SB_PER_IT
            if it == 0:
                # JIT v-prologue: group sbj's vt rows are consumed by this
                # superburst's AV matmuls
                for jblk in range(4 * sbj, 4 * (sbj + 1)):
                    emit_v(jblk)
            if pending:
                pending.pop(0)()
            if sbj == 0:
                av_t = [avp.tile([128, 512], FP32, tag="av",
                                 name=f"av{it}_{bk}") for bk in range(3)]
            emit_av(s, pts.pop(s), av_t)
            if s + 1 < N_SB:
                pts[s + 1] = emit_qk(s + 1)
            if sbj == SB_PER_IT - 1:
                for cc in range(2):
                    avc[(it, cc)] = avcp.tile([128, IT], FP16, tag=f"avc{cc}",
                                              name=f"avc{it}_{cc}")
                at = av_t
                # all four chunk_a's drain now: the next i-tile's first AV
                # matmul clears whole banks, so every av_t read must precede
                # the av slot handover
                for ic in range(4):
                    epi_chunk_a(it, ic, at)
                pending.extend([
                    (lambda it=it: epi_chunk_b(it, 0, 0)),
                    (lambda it=it: epi_chunk_b(it, 0, 1)),
                    (lambda it=it: epi_chunk_b(it, 1, 0)),
                    (lambda it=it: epi_chunk_b(it, 1, 1)),
                ])
        while pending:
            pending.pop(0)()

        # ---- cross-core InstanceNorm stats -----------------------------
        mv2 = persist.tile([128, 2, 2], FP32, tag="mv2", name="mv2")
        if use_collective:
            nc.gpsimd.collective_compute(
                "AllGather", ALU.bypass, replica_groups=REPLICA_GROUPS,
                ins=[ccin_dr.opt()], outs=[ccout_dr.opt()])
            cc16 = persist.tile([128, 2, N_ITILES, 2, 6], FP32, tag="cc16",
                                name="cc16")
            for r in range(2):
                nc.sync.dma_start(cc16[:, r], ccout_dr[r])
            for ob in range(2):
                nc.vector.bn_aggr(
                    out=mv2[:, ob],
                    in_=cc16.rearrange("p r i o s -> p o (r i) s")[:, ob])
        else:
            for ob in range(2):
                nc.vector.bn_aggr(out=mv2[:, ob], in_=stats_sb[ob])

        # rstd = 1/sqrt(var + eps); negmr = -mean * rstd
        rstd2 = persist.tile([128, 2], FP32, tag="rstd2", name="rstd2")
        nc.scalar.activation(out=rstd2, in_=mv2[:, :, 1], func=AF.Sqrt,
                             bias=eps_sb, scale=1.0)
        nc.vector.reciprocal(rstd2, rstd2)
        negmr = persist.tile([128, 2], FP32, tag="negmr", name="negmr")
        nc.vector.tensor_mul(negmr, mv2[:, :, 0], rstd2)
        nc.vector.tensor_scalar_mul(negmr, in0=negmr, scalar1=-1.0)

        # ---- final norm + residual + store -----------------------------
        stq = [nc.sync, nc.gpsimd]
        for ob in range(2):
            rstd = rstd2[:, ob:ob + 1]
            nmr = negmr[:, ob:ob + 1]
            for ch in range(4):
                sl = slice(512 * ch, 512 * (ch + 1))
                nc.scalar.activation(
                    out=proj_sb[ob][:, sl], in_=proj_sb[ob][:, sl],
                    func=AF.Identity, bias=nmr, scale=rstd)
                nc.vector.tensor_add(proj_sb[ob][:, sl], proj_sb[ob][:, sl],
                                     x1_sb[ob][:, sl])
                stq[(ob * 4 + ch) % 2].dma_start(
                    out_d[128 * ob:128 * (ob + 1), sl], proj_sb[ob][:, sl])


_nc_cache = None


def _get_nc():
    global _nc_cache
    if _nc_cache is None:
        import os
        _nc_cache = build_nc(
            use_collective=not os.environ.get("XATTN_NO_COLLECTIVE"))
    return _nc_cache


def make_in_maps(x1, x2, wq, wk, wv, wp):
    x1f = np.asarray(x1, np.float32).reshape(B, C, N).astype(np.float16)
    x2f = np.asarray(x2, np.float32).reshape(B, C, N).astype(np.float16)
    wvT = np.asarray(wv, np.float32).T
    wkT = np.tile(np.asarray(wk, np.float32).T, (1, 4))
    wqT = np.tile(np.asarray(wq, np.float32).T, (1, 4))
    wpT = np.asarray(wp, np.float32).T
    wcat = np.ascontiguousarray(
        np.concatenate([wvT, wkT, wqT, wpT], axis=1).astype(np.float16))
    in_maps = []
    for core in range(N_CORES):
        b, h = core // 2, core % 2
        in_maps.append({
            "x1h": np.ascontiguousarray(x1f[b, :, h * NH:(h + 1) * NH]),
            "x2b": np.ascontiguousarray(x2f[b]),
            "wcat": wcat,
        })
    return in_maps


def assemble_out(results):
    out = np.empty((B, C, N), np.float32)
    for core in range(N_CORES):
        b, h = core // 2, core % 2
        out[b, :, h * NH:(h + 1) * NH] = results[core]["out"]
    return out.reshape(B, C, 16, 16, 16)


def kernel(**inputs):
    global LAST_RESULTS
    in_maps = make_in_maps(inputs["x1"], inputs["x2"], inputs["wq"],
                           inputs["wk"], inputs["wv"], inputs["wp"])
    res = run_bass_kernel_spmd(_get_nc(), in_maps, core_ids=list(range(N_CORES)))
    LAST_RESULTS = res
    return assemble_out(res.results)


# revision 22
# speedup vs baseline: 1.2653x; 1.2653x over previous
"""Trainium2 Bass kernel for nn_CrossAttention (single-head NxN attention + proj + InstanceNorm + residual).

Sharding: 8 cores = (batch b in 0..3) x (query-half h in 0..1).
Each core computes its half of the query tokens for one batch; the
InstanceNorm statistics (over the full 4096 tokens) are combined across
the core pair with a tiny AllGather (a dummy warmup AllGather at kernel
start absorbs the ~45us first-collective cost).

v3 structure:
 - PE clock warmup: dummy matmuls during the input-DMA prologue keep the
   PE HAM activity monitor busy so the first real matmuls run at 2.4 GHz.
 - Coarse input DMA: weights are host-concatenated into one tensor and
   the whole input set moves in 8 large descriptors (descriptor issue
   costs ~0.6us each and rings only keep ~2 in flight).
 - i-tiles of 512 queries: QK runs as 4 concurrent 32-row-group matmuls
   (one per j-block) with 512-wide moving operands -- ~3x fewer PE
   cycles than 256-wide 2-way-concurrent tiles.  One exp() activation
   per superburst covers [128, 2048] (less ACT overhead).
 - PSUM: qk [128,4,512] (4 banks) + 3 av accumulator banks holding 8
   packed regions (per 128-query chunk: [c0..127+denom] and [c128..255])
   + 1 bank for transpose/proj/v-prologue tiles.
 - Raw per-i-tile bn_stats are exchanged in the AllGather, staged into
   the collective input DRAM as produced; the pair combination is two
   bn_aggr calls.

Precision: fp16 matmul operands everywhere, fp32 PSUM accumulation; the
qk*Cr^-0.5 scale is folded into the exp() activation scale.

Self-contained: hardcodes shapes B=4, C=256, D=H=W=16 (N=4096), Cr=32.
"""

import numpy as np

import concourse.bass as bass
import concourse.mybir as mybir
import concourse.tile as tile
from concourse import bacc
from concourse.bass_utils import run_bass_kernel_spmd
from concourse.masks import make_identity

B, C, N, Cr = 4, 256, 4096, 32
NH = N // 2  # query tokens per core
EPS = 1e-5
SCALE = float(Cr) ** -0.5
FP32 = mybir.dt.float32
FP16 = mybir.dt.float16

N_CORES = 8
REPLICA_GROUPS = [[0, 1], [2, 3], [4, 5], [6, 7]]

IT = 512                   # i-tile width (query columns per superburst)
N_ITILES = NH // IT        # 4
JBLK = 128                 # j-block (rows per QK matmul output)
N_JBLK = N // JBLK         # 32
JB_PER_SB = 2              # j-blocks per superburst (2-way row-tiled QK)
SB_PER_IT = N_JBLK // JB_PER_SB  # 16
N_SB = N_ITILES * SB_PER_IT      # 64

# av accumulator packing: 8 regions (4 query-chunks x {A: c0-127+denom,
# B: c128-255}) packed 3 per PSUM bank at 136-col stride
AV_REGION_STRIDE = 136

N_WARM_MM = 8              # dummy matmuls to warm the PE clock gate

AF = mybir.ActivationFunctionType
ALU = mybir.AluOpType

LAST_RESULTS = None  # BassKernelResults of the most recent run (for test harness)


def build_nc(use_collective=True):
    nc = bacc.Bacc("TRN2", num_devices=N_CORES, name="xattn",
                   target_bir_lowering=False)

    x1h_d = nc.dram_tensor("x1h", [C, NH], FP16, kind="ExternalInput").ap()
    x2b_d = nc.dram_tensor("x2b", [C, N], FP16, kind="ExternalInput").ap()
    # wv[256] | wk[128] | wq[128] | wp[256] concatenated along the free dim
    wcat_d = nc.dram_tensor("wcat", [C, 768], FP16, kind="ExternalInput").ap()
    out_d = nc.dram_tensor("out", [C, NH], FP32, kind="ExternalOutput").ap()

    with tile.TileContext(nc) as tc:
        build_body(tc, x1h_d, x2b_d, wcat_d, out_d, use_collective)
    nc.compile()
    return nc


def build_body(tc, x1h_d, x2b_d, wcat_d, out_d, use_collective=True):
    nc = tc.nc
    from contextlib import ExitStack

    with ExitStack() as ctx:
        persist = ctx.enter_context(tc.tile_pool(name="persist", bufs=1))
        sm = ctx.enter_context(tc.tile_pool(name="sm", bufs=4))
        avcp = ctx.enter_context(tc.tile_pool(name="avcp", bufs=2))
        ptp = ctx.enter_context(tc.tile_pool(name="ptp", bufs=3))
        qkp = ctx.enter_context(tc.tile_pool(name="qkp", bufs=2, space="PSUM"))
        avp = ctx.enter_context(tc.tile_pool(name="avp", bufs=3, space="PSUM"))
        # one bank shared by the prologue vp tiles and the epilogue tp/pj
        # tiles (PSUM pool slots are bank-rounded, so bufs=1)
        epi = ctx.enter_context(tc.tile_pool(name="epi", bufs=1, space="PSUM"))
        dramp = ctx.enter_context(tc.tile_pool(name="dramp", bufs=1, space="DRAM"))

        # ---- PE clock warmup: dummy matmuls on a memset tile ------------
        warm_mm = persist.tile([128, 512], FP16, tag="warm_mm", name="warm_mm")
        nc.vector.memset(warm_mm, 0.0)
        warm_ps = qkp.tile([128, 512], FP32, tag="qk", name="warm_ps")
        for w in range(N_WARM_MM):
            nc.tensor.matmul(warm_ps, lhsT=warm_mm[:, 0:128], rhs=warm_mm,
                             start=True, stop=True)

        # ---- warmup collective ------------------------------------------
        if use_collective:
            warm_sb = persist.tile([128, 4], FP32, tag="warm", name="warm_sb")
            nc.vector.memset(warm_sb, 0.0)
            warm_in = dramp.tile([128, 4], FP32, tag="warm_i", name="warm_in")
            warm_out = dramp.tile([2, 128, 4], FP32, tag="warm_o", name="warm_out")
            nc.sync.dma_start(warm_in, warm_sb)
            nc.gpsimd.collective_compute(
                "AllGather", ALU.bypass, replica_groups=REPLICA_GROUPS,
                ins=[warm_in.opt()], outs=[warm_out.opt()])

        # ---- constants -------------------------------------------------
        eps_sb = persist.tile([128, 1], FP32, tag="eps", name="eps_sb")
        nc.vector.memset(eps_sb, EPS)
        ident = persist.tile([128, 128], FP16, tag="ident", name="ident")
        make_identity(nc, ident)

        # ---- input DMAs (large descriptors, weights first) -------------
        wcat_sb = [persist.tile([128, 768], FP16, tag=f"wcat{cc}",
                                name=f"wcat_sb{cc}") for cc in range(2)]
        wv_sb = [wcat_sb[cc][:, 0:256] for cc in range(2)]
        wk_sb = [wcat_sb[cc][:, 256:384] for cc in range(2)]
        wq_sb = [wcat_sb[cc][:, 384:512] for cc in range(2)]
        wp_sb = [wcat_sb[cc][:, 512:768] for cc in range(2)]
        x2_sb = [persist.tile([128, N], FP16, tag=f"x2_{cc}", name=f"x2_sb{cc}")
                 for cc in range(2)]
        x1_sb = [persist.tile([128, NH], FP16, tag=f"x1_{cc}", name=f"x1_sb{cc}")
                 for cc in range(2)]
        for cc in range(2):
            sl = slice(128 * cc, 128 * (cc + 1))
            nc.scalar.dma_start(wcat_sb[cc], wcat_d[sl, :])
        x2q = [nc.sync, nc.gpsimd]
        for ch in range(2):
            slh = slice(2048 * ch, 2048 * (ch + 1))
            for cc in range(2):
                x2q[cc].dma_start(x2_sb[cc][:, slh],
                                  x2b_d[128 * cc:128 * (cc + 1), slh])
        for cc in range(2):
            nc.scalar.dma_start(x1_sb[cc], x1h_d[128 * cc:128 * (cc + 1), :])

        # ---- prologue: vt / k_rep / q_rep ------------------------------
        # vt[j, :] = [v(c0..127) | ones | v(c128..255)] per j-block; the
        # ones column rides the A-half AV matmul as the softmax denominator
        vt = persist.tile([128, N_JBLK, C + 1], FP16, tag="vt", name="vt")
        nc.vector.memset(vt[:, :, 128:129], 1.0)
        k_rep = persist.tile([128, N], FP16, tag="krep", name="k_rep")
        q_rep = persist.tile([128, NH], FP16, tag="qrep", name="q_rep")

        def emit_v(jblk):
            vp = epi.tile([128, C], FP32, tag="epi", name=f"vp{jblk}")
            for cc in range(2):
                nc.tensor.matmul(
                    vp, lhsT=x2_sb[cc][:, 128 * jblk:128 * (jblk + 1)],
                    rhs=wv_sb[cc], start=(cc == 0), stop=(cc == 1))
            if jblk % 2 == 0:
                nc.vector.tensor_copy(vt[:, jblk, 0:128], vp[:, 0:128])
                nc.scalar.copy(vt[:, jblk, 129:257], vp[:, 128:256])
            else:
                nc.scalar.copy(vt[:, jblk, 0:128], vp[:, 0:128])
                nc.vector.tensor_copy(vt[:, jblk, 129:257], vp[:, 128:256])

        def emit_kq(which, u):
            w_sb, dst = (wk_sb, k_rep) if which == "k" else (wq_sb, q_rep)
            src = x2_sb if which == "k" else x1_sb
            kp = qkp.tile([128, 512], FP32, tag="qk", name=f"{which}p{u}")
            for cc in range(2):
                nc.tensor.matmul(
                    kp, lhsT=w_sb[cc],
                    rhs=src[cc][:, 512 * u:512 * (u + 1)],
                    start=(cc == 0), stop=(cc == 1))
            nc.vector.tensor_copy(dst[:, 512 * u:512 * (u + 1)], kp)

        # k/q prologue (x2 chunk 0 covers k0-3, chunk 1 covers k4-7)
        for u in range(4):
            emit_kq("k", u)
            emit_kq("q", u)
        for u in range(4, 8):
            emit_kq("k", u)

        # ---- persistent attention outputs ------------------------------
        proj_sb = [persist.tile([128, NH], FP32, tag=f"proj{ob}",
                                name=f"proj_sb{ob}") for ob in range(2)]
        stats_sb = [persist.tile([128, N_ITILES, 6], FP32, tag=f"stats{ob}",
                                 name=f"stats_sb{ob}") for ob in range(2)]
        if use_collective:
            ccin_dr = dramp.tile([128, N_ITILES, 2, 6], FP32, tag="ccin_d",
                                 name="ccin_dr")
            ccout_dr = dramp.tile([2, 128, N_ITILES, 2, 6], FP32,
                                  tag="ccout_d", name="ccout_dr")

        # ---- main loop: 32 superbursts of (4-way QK, one exp, AV) ------
        def emit_qk(s):
            it, sbj = s // SB_PER_IT, s % SB_PER_IT
            isl = slice(IT * it, IT * (it + 1))
            qk = qkp.tile([128, JB_PER_SB, IT], FP32, tag="qk", name=f"qk{s}")
            # alternate row-group pairs per superburst so the next burst's
            # LDWEIGHTS can pull ahead of in-flight matmuls
            gb = (sbj % 2) * 2
            for g in range(JB_PER_SB):
                jblk = sbj * JB_PER_SB + g
                rg = 32 * (gb + g)
                nc.tensor.matmul(
                    qk[:, g, :],
                    lhsT=k_rep[rg:rg + 32,
                               JBLK * jblk:JBLK * (jblk + 1)],
                    rhs=q_rep[rg:rg + 32, isl],
                    start=True, stop=True, tile_position=(rg, 0))
            pt = ptp.tile([128, JB_PER_SB, IT], FP16, tag="pt", name=f"pt{s}")
            nc.scalar.activation(out=pt, in_=qk, func=AF.Exp, scale=SCALE)
            return pt

        def av_region(av_t, rid):
            bank, slot = rid // 3, rid % 3
            w = 129 if rid % 2 == 0 else 128
            off = AV_REGION_STRIDE * slot
            return av_t[bank][:, off:off + w]

        def emit_av(s, pt, av_t):
            it, sbj = s // SB_PER_IT, s % SB_PER_IT
            first = sbj == 0
            last = sbj == SB_PER_IT - 1
            for g in range(JB_PER_SB):
                jblk = sbj * JB_PER_SB + g
                for ic in range(IT // 128):
                    lhsT = pt[:, g, 128 * ic:128 * (ic + 1)]
                    for ab in range(2):
                        rid = ic * 2 + ab
                        rhs = (vt[:, jblk, 0:129] if ab == 0
                               else vt[:, jblk, 129:257])
                        nc.tensor.matmul(
                            av_region(av_t, rid), lhsT=lhsT, rhs=rhs,
                            # start=True clears the has_written bits of the
                            # whole bank: only the first region per bank may
                            # set it (rids 0,3,6 are first in banks 0,1,2);
                            # the other regions' first writes overwrite via
                            # unset bits.
                            start=(first and g == 0 and rid in (0, 3, 6)),
                            stop=(last and g == JB_PER_SB - 1))

        avc = {}

        def epi_chunk_a(it, ic, av_t):
            a = av_region(av_t, ic * 2)
            b = av_region(av_t, ic * 2 + 1)
            rden = sm.tile([128, 1], FP32, tag="rden", name=f"rden{it}_{ic}")
            nc.vector.reciprocal(rden, a[:, 128:129])
            avn = sm.tile([128, C], FP16, tag="avn", name=f"avn{it}_{ic}")
            nc.vector.tensor_scalar_mul(avn[:, 0:128], in0=a[:, 0:128],
                                        scalar1=rden)
            nc.vector.tensor_scalar_mul(avn[:, 128:256], in0=b, scalar1=rden)
            tp = epi.tile([128, C], FP16, tag="epi", name=f"tp{it}_{ic}")
            nc.tensor.transpose(tp[:, 0:128], avn[:, 0:128], ident)
            nc.tensor.transpose(tp[:, 128:256], avn[:, 128:256], ident)
            for cc in range(2):
                nc.vector.tensor_copy(
                    avc[(it, cc)][:, 128 * ic:128 * (ic + 1)],
                    tp[:, 128 * cc:128 * (cc + 1)])

        def epi_chunk_b(it, ob, ih):
            isl = slice(IT * it + 256 * ih, IT * it + 256 * (ih + 1))
            pj = epi.tile([128, 256], FP32, tag="epi", name=f"pj{it}_{ob}_{ih}")
            for cc in range(2):
                nc.tensor.matmul(
                    pj, lhsT=wp_sb[cc][:, 128 * ob:128 * (ob + 1)],
                    rhs=avc[(it, cc)][:, 256 * ih:256 * (ih + 1)],
                    start=(cc == 0), stop=(cc == 1))
            nc.vector.tensor_copy(proj_sb[ob][:, isl], pj)
            if ih == 1:
                tsl = slice(IT * it, IT * (it + 1))
                nc.vector.bn_stats(stats_sb[ob][:, it, :], proj_sb[ob][:, tsl])
                if use_collective:
                    nc.sync.dma_start(ccin_dr[:, it, ob],
                                      stats_sb[ob][:, it, :])

        pending = []
        pts = {0: emit_qk(0), 1: emit_qk(1)}
        av_t = None
        for s in range(N_SB):
            it, sbj = s // SB_PER_IT, s % SB_PER_IT
            if it == 0:
                # JIT v-prologue: group sbj's vt rows are consumed by this
                # superburst's AV matmuls
                for jblk in range(JB_PER_SB * sbj, JB_PER_SB * (sbj + 1)):
                    emit_v(jblk)
            if pending:
                pending.pop(0)()
            if sbj == 0:
                av_t = [avp.tile([128, 512], FP32, tag="av",
                                 name=f"av{it}_{bk}") for bk in range(3)]
            # QK(s+2) is emitted before AV(s): both wait on exp(s) (PSUM
            # buffer WAR / pt data), and the PE must not sit behind AV in
            # the queue when exp(s+1) depends on QK(s+1)
            if s + 2 < N_SB:
                pts[s + 2] = emit_qk(s + 2)
            emit_av(s, pts.pop(s), av_t)
            if sbj == SB_PER_IT - 1:
                for cc in range(2):
                    avc[(it, cc)] = avcp.tile([128, IT], FP16, tag=f"avc{cc}",
                                              name=f"avc{it}_{cc}")
                at = av_t
                # all four chunk_a's drain now: the next i-tile's first AV
                # matmul clears whole banks, so every av_t read must precede
                # the av slot handover
                for ic in range(4):
                    epi_chunk_a(it, ic, at)
                pending.extend([
                    (lambda it=it: epi_chunk_b(it, 0, 0)),
                    (lambda it=it: epi_chunk_b(it, 0, 1)),
                    (lambda it=it: epi_chunk_b(it, 1, 0)),
                    (lambda it=it: epi_chunk_b(it, 1, 1)),
                ])
        while pending:
            pending.pop(0)()

        # ---- cross-core InstanceNorm stats -----------------------------
        mv2 = persist.tile([128, 2, 2], FP32, tag="mv2", name="mv2")
        if use_collective:
            nc.gpsimd.collective_compute(
                "AllGather", ALU.bypass, replica_groups=REPLICA_GROUPS,
                ins=[ccin_dr.opt()], outs=[ccout_dr.opt()])
            cc16 = persist.tile([128, 2, N_ITILES, 2, 6], FP32, tag="cc16",
                                name="cc16")
            for r in range(2):
                nc.sync.dma_start(cc16[:, r], ccout_dr[r])
            for ob in range(2):
                nc.vector.bn_aggr(
                    out=mv2[:, ob],
                    in_=cc16.rearrange("p r i o s -> p o (r i) s")[:, ob])
        else:
            for ob in range(2):
                nc.vector.bn_aggr(out=mv2[:, ob], in_=stats_sb[ob])

        # rstd = 1/sqrt(var + eps); negmr = -mean * rstd
        rstd2 = persist.tile([128, 2], FP32, tag="rstd2", name="rstd2")
        nc.scalar.activation(out=rstd2, in_=mv2[:, :, 1], func=AF.Sqrt,
                             bias=eps_sb, scale=1.0)
        nc.vector.reciprocal(rstd2, rstd2)
        negmr = persist.tile([128, 2], FP32, tag="negmr", name="negmr")
        nc.vector.tensor_mul(negmr, mv2[:, :, 0], rstd2)
        nc.vector.tensor_scalar_mul(negmr, in0=negmr, scalar1=-1.0)

        # ---- final norm + residual + store -----------------------------
        stq = [nc.sync, nc.gpsimd]
        for ob in range(2):
            rstd = rstd2[:, ob:ob + 1]
            nmr = negmr[:, ob:ob + 1]
            for ch in range(4):
                sl = slice(512 * ch, 512 * (ch + 1))
                nc.scalar.activation(
                    out=proj_sb[ob][:, sl], in_=proj_sb[ob][:, sl],
                    func=AF.Identity, bias=nmr, scale=rstd)
                nc.vector.tensor_add(proj_sb[ob][:, sl], proj_sb[ob][:, sl],
                                     x1_sb[ob][:, sl])
                stq[(ob * 4 + ch) % 2].dma_start(
                    out_d[128 * ob:128 * (ob + 1), sl], proj_sb[ob][:, sl])


_nc_cache = None


def _get_nc():
    global _nc_cache
    if _nc_cache is None:
        import os
        _nc_cache = build_nc(
            use_collective=not os.environ.get("XATTN_NO_COLLECTIVE"))
    return _nc_cache


def make_in_maps(x1, x2, wq, wk, wv, wp):
    x1f = np.asarray(x1, np.float32).reshape(B, C, N).astype(np.float16)
    x2f = np.asarray(x2, np.float32).reshape(B, C, N).astype(np.float16)
    wvT = np.asarray(wv, np.float32).T
    wkT = np.tile(np.asarray(wk, np.float32).T, (1, 4))
    wqT = np.tile(np.asarray(wq, np.float32).T, (1, 4))
    wpT = np.asarray(wp, np.float32).T
    wcat = np.ascontiguousarray(
        np.concatenate([wvT, wkT, wqT, wpT], axis=1).astype(np.float16))
    in_maps = []
    for core in range(N_CORES):
        b, h = core // 2, core % 2
        in_maps.append({
            "x1h": np.ascontiguousarray(x1f[b, :, h * NH:(h + 1) * NH]),
            "x2b": np.ascontiguousarray(x2f[b]),
            "wcat": wcat,
        })
    return in_maps


def assemble_out(results):
    out = np.empty((B, C, N), np.float32)
    for core in range(N_CORES):
        b, h = core // 2, core % 2
        out[b, :, h * NH:(h + 1) * NH] = results[core]["out"]
    return out.reshape(B, C, 16, 16, 16)


def kernel(**inputs):
    global LAST_RESULTS
    in_maps = make_in_maps(inputs["x1"], inputs["x2"], inputs["wq"],
                           inputs["wk"], inputs["wv"], inputs["wp"])
    res = run_bass_kernel_spmd(_get_nc(), in_maps, core_ids=list(range(N_CORES)))
    LAST_RESULTS = res
    return assemble_out(res.results)


# revision 30
# speedup vs baseline: 1.2917x; 1.0208x over previous
"""Trainium2 Bass kernel for nn_CrossAttention (single-head NxN attention + proj + InstanceNorm + residual).

Sharding: 8 cores = (batch b in 0..3) x (query-half h in 0..1).
Each core computes its half of the query tokens for one batch; the
InstanceNorm statistics (over the full 4096 tokens) are combined across
the core pair with a tiny AllGather (a dummy warmup AllGather at kernel
start absorbs the ~45us first-collective cost).

v3 structure:
 - PE clock warmup: dummy matmuls during the input-DMA prologue keep the
   PE HAM activity monitor busy so the first real matmuls run at 2.4 GHz.
 - Coarse input DMA: weights are host-concatenated into one tensor and
   the whole input set moves in 8 large descriptors (descriptor issue
   costs ~0.6us each and rings only keep ~2 in flight).
 - i-tiles of 512 queries: QK runs as 4 concurrent 32-row-group matmuls
   (one per j-block) with 512-wide moving operands -- ~3x fewer PE
   cycles than 256-wide 2-way-concurrent tiles.  One exp() activation
   per superburst covers [128, 2048] (less ACT overhead).
 - PSUM: qk [128,4,512] (4 banks) + 3 av accumulator banks holding 8
   packed regions (per 128-query chunk: [c0..127+denom] and [c128..255])
   + 1 bank for transpose/proj/v-prologue tiles.
 - Raw per-i-tile bn_stats are exchanged in the AllGather, staged into
   the collective input DRAM as produced; the pair combination is two
   bn_aggr calls.

Precision: fp16 matmul operands everywhere, fp32 PSUM accumulation; the
qk*Cr^-0.5 scale is folded into the exp() activation scale.

Self-contained: hardcodes shapes B=4, C=256, D=H=W=16 (N=4096), Cr=32.
"""

import numpy as np

import concourse.bass as bass
import concourse.mybir as mybir
import concourse.tile as tile
from concourse import bacc
from concourse.bass_utils import run_bass_kernel_spmd
from concourse.masks import make_identity

B, C, N, Cr = 4, 256, 4096, 32
NH = N // 2  # query tokens per core
EPS = 1e-5
SCALE = float(Cr) ** -0.5
FP32 = mybir.dt.float32
FP16 = mybir.dt.float16

N_CORES = 8
REPLICA_GROUPS = [[0, 1], [2, 3], [4, 5], [6, 7]]

IT = 512                   # i-tile width (query columns per superburst)
N_ITILES = NH // IT        # 4
JBLK = 128                 # j-block (rows per QK matmul output)
N_JBLK = N // JBLK         # 32
JB_PER_SB = 2              # j-blocks per superburst (2-way row-tiled QK)
SB_PER_IT = N_JBLK // JB_PER_SB  # 16
N_SB = N_ITILES * SB_PER_IT      # 64

# av accumulator packing: 8 regions (4 query-chunks x {A: c0-127+denom,
# B: c128-255}) packed 3 per PSUM bank at 136-col stride
AV_REGION_STRIDE = 136

N_WARM_MM = 8              # dummy matmuls to warm the PE clock gate

AF = mybir.ActivationFunctionType
ALU = mybir.AluOpType

LAST_RESULTS = None  # BassKernelResults of the most recent run (for test harness)


def build_nc(use_collective=True):
    nc = bacc.Bacc("TRN2", num_devices=N_CORES, name="xattn",
                   target_bir_lowering=False)

    x1h_d = nc.dram_tensor("x1h", [C, NH], FP16, kind="ExternalInput").ap()
    x2b_d = nc.dram_tensor("x2b", [C, N], FP16, kind="ExternalInput").ap()
    # wv[256] | wk[128] | wq[128] | wp[256] concatenated along the free dim
    wcat_d = nc.dram_tensor("wcat", [C, 768], FP16, kind="ExternalInput").ap()
    out_d = nc.dram_tensor("out", [C, NH], FP32, kind="ExternalOutput").ap()

    with tile.TileContext(nc) as tc:
        build_body(tc, x1h_d, x2b_d, wcat_d, out_d, use_collective)
    nc.compile()
    return nc


def build_body(tc, x1h_d, x2b_d, wcat_d, out_d, use_collective=True):
    nc = tc.nc
    from contextlib import ExitStack

    with ExitStack() as ctx:
        persist = ctx.enter_context(tc.tile_pool(name="persist", bufs=1))
        sm = ctx.enter_context(tc.tile_pool(name="sm", bufs=4))
        avcp = ctx.enter_context(tc.tile_pool(name="avcp", bufs=2))
        ptp = ctx.enter_context(tc.tile_pool(name="ptp", bufs=3))
        qkp = ctx.enter_context(tc.tile_pool(name="qkp", bufs=2, space="PSUM"))
        avp = ctx.enter_context(tc.tile_pool(name="avp", bufs=3, space="PSUM"))
        # one bank shared by the prologue vp tiles and the epilogue tp/pj
        # tiles (PSUM pool slots are bank-rounded, so bufs=1)
        epi = ctx.enter_context(tc.tile_pool(name="epi", bufs=1, space="PSUM"))
        dramp = ctx.enter_context(tc.tile_pool(name="dramp", bufs=1, space="DRAM"))

        # ---- PE clock warmup: dummy matmuls on a memset tile ------------
        warm_mm = persist.tile([128, 512], FP16, tag="warm_mm", name="warm_mm")
        nc.vector.memset(warm_mm, 0.0)
        warm_ps = qkp.tile([128, 512], FP32, tag="qk", name="warm_ps")
        for w in range(N_WARM_MM):
            nc.tensor.matmul(warm_ps, lhsT=warm_mm[:, 0:128], rhs=warm_mm,
                             start=True, stop=True)

        # ---- warmup collective ------------------------------------------
        if use_collective:
            warm_sb = persist.tile([128, 4], FP32, tag="warm", name="warm_sb")
            nc.vector.memset(warm_sb, 0.0)
            warm_in = dramp.tile([128, 4], FP32, tag="warm_i", name="warm_in")
            warm_out = dramp.tile([2, 128, 4], FP32, tag="warm_o", name="warm_out")
            nc.sync.dma_start(warm_in, warm_sb)
            nc.gpsimd.collective_compute(
                "AllGather", ALU.bypass, replica_groups=REPLICA_GROUPS,
                ins=[warm_in.opt()], outs=[warm_out.opt()])

        # ---- constants -------------------------------------------------
        eps_sb = persist.tile([128, 1], FP32, tag="eps", name="eps_sb")
        nc.vector.memset(eps_sb, EPS)
        ident = persist.tile([128, 128], FP16, tag="ident", name="ident")
        make_identity(nc, ident)

        # ---- input DMAs (large descriptors, weights first) -------------
        wcat_sb = [persist.tile([128, 768], FP16, tag=f"wcat{cc}",
                                name=f"wcat_sb{cc}") for cc in range(2)]
        wv_sb = [wcat_sb[cc][:, 0:256] for cc in range(2)]
        wk_sb = [wcat_sb[cc][:, 256:384] for cc in range(2)]
        wq_sb = [wcat_sb[cc][:, 384:512] for cc in range(2)]
        wp_sb = [wcat_sb[cc][:, 512:768] for cc in range(2)]
        x2_sb = [persist.tile([128, N], FP16, tag=f"x2_{cc}", name=f"x2_sb{cc}")
                 for cc in range(2)]
        x1_sb = [persist.tile([128, NH], FP16, tag=f"x1_{cc}", name=f"x1_sb{cc}")
                 for cc in range(2)]
        for cc in range(2):
            sl = slice(128 * cc, 128 * (cc + 1))
            nc.scalar.dma_start(wcat_sb[cc], wcat_d[sl, :])
        x2q = [nc.sync, nc.gpsimd]
        for ch in range(2):
            slh = slice(2048 * ch, 2048 * (ch + 1))
            for cc in range(2):
                x2q[cc].dma_start(x2_sb[cc][:, slh],
                                  x2b_d[128 * cc:128 * (cc + 1), slh])
        for cc in range(2):
            nc.scalar.dma_start(x1_sb[cc], x1h_d[128 * cc:128 * (cc + 1), :])

        # ---- prologue: vt / k_rep / q_rep ------------------------------
        # vt[j, :] = [v(c0..127) | ones | v(c128..255)] per j-block; the
        # ones column rides the A-half AV matmul as the softmax denominator
        vt = persist.tile([128, N_JBLK, C + 1], FP16, tag="vt", name="vt")
        nc.vector.memset(vt[:, :, 128:129], 1.0)
        k_rep = persist.tile([128, N], FP16, tag="krep", name="k_rep")
        q_rep = persist.tile([128, NH], FP16, tag="qrep", name="q_rep")

        def emit_v(jblk):
            vp = epi.tile([128, C], FP32, tag="epi", name=f"vp{jblk}")
            for cc in range(2):
                nc.tensor.matmul(
                    vp, lhsT=x2_sb[cc][:, 128 * jblk:128 * (jblk + 1)],
                    rhs=wv_sb[cc], start=(cc == 0), stop=(cc == 1))
            if jblk % 2 == 0:
                nc.vector.tensor_copy(vt[:, jblk, 0:128], vp[:, 0:128])
                nc.scalar.copy(vt[:, jblk, 129:257], vp[:, 128:256])
            else:
                nc.scalar.copy(vt[:, jblk, 0:128], vp[:, 0:128])
                nc.vector.tensor_copy(vt[:, jblk, 129:257], vp[:, 128:256])

        def emit_kq(which, u):
            w_sb, dst = (wk_sb, k_rep) if which == "k" else (wq_sb, q_rep)
            src = x2_sb if which == "k" else x1_sb
            kp = qkp.tile([128, 512], FP32, tag="qk", name=f"{which}p{u}")
            for cc in range(2):
                nc.tensor.matmul(
                    kp, lhsT=w_sb[cc],
                    rhs=src[cc][:, 512 * u:512 * (u + 1)],
                    start=(cc == 0), stop=(cc == 1))
            nc.vector.tensor_copy(dst[:, 512 * u:512 * (u + 1)], kp)

        # k/q/v prologue, interleaved so the vp/kp copy-latency chains hide
        # behind each other's matmuls (x2 chunk 0 covers k0-3/v0-15)
        for u in range(8):
            emit_kq("k", u)
            if u < 4:
                emit_kq("q", u)
            for jblk in range(4 * u, 4 * (u + 1)):
                emit_v(jblk)

        # ---- persistent attention outputs ------------------------------
        proj_sb = [persist.tile([128, NH], FP32, tag=f"proj{ob}",
                                name=f"proj_sb{ob}") for ob in range(2)]
        # [ob, it, ih, 6] layout: (r, it, ih) merges into one bn_aggr axis
        stats_sb = persist.tile([128, 2, N_ITILES, 2, 6], FP32, tag="stats",
                                name="stats_sb")
        # av accumulators are drained to SBUF in one fast copy per i-tile so
        # the next i-tile's AV matmuls only wait ~1us, not the whole epilogue
        av_stage = persist.tile([128, 3, 512], FP32, tag="avst",
                                name="av_stage")
        if use_collective:
            ccin_dr = dramp.tile([128, 2, N_ITILES, 2, 6], FP32, tag="ccin_d",
                                 name="ccin_dr")
            ccout_dr = dramp.tile([2, 128, 2, N_ITILES, 2, 6], FP32,
                                  tag="ccout_d", name="ccout_dr")

        # ---- main loop: 32 superbursts of (4-way QK, one exp, AV) ------
        def emit_qk(s):
            it, sbj = s // SB_PER_IT, s % SB_PER_IT
            isl = slice(IT * it, IT * (it + 1))
            qk = qkp.tile([128, JB_PER_SB, IT], FP32, tag="qk", name=f"qk{s}")
            # alternate row-group pairs per superburst so the next burst's
            # LDWEIGHTS can pull ahead of in-flight matmuls
            gb = (sbj % 2) * 2
            for g in range(JB_PER_SB):
                jblk = sbj * JB_PER_SB + g
                rg = 32 * (gb + g)
                nc.tensor.matmul(
                    qk[:, g, :],
                    lhsT=k_rep[rg:rg + 32,
                               JBLK * jblk:JBLK * (jblk + 1)],
                    rhs=q_rep[rg:rg + 32, isl],
                    start=True, stop=True, tile_position=(rg, 0))
            pt = ptp.tile([128, JB_PER_SB, IT], FP16, tag="pt", name=f"pt{s}")
            nc.scalar.activation(out=pt, in_=qk, func=AF.Exp, scale=SCALE)
            return pt

        def av_region(av_t, rid):
            bank, slot = rid // 3, rid % 3
            w = 129 if rid % 2 == 0 else 128
            off = AV_REGION_STRIDE * slot
            return av_t[bank][:, off:off + w]

        def emit_av(s, pt, av_t):
            it, sbj = s // SB_PER_IT, s % SB_PER_IT
            first = sbj == 0
            last = sbj == SB_PER_IT - 1
            for g in range(JB_PER_SB):
                jblk = sbj * JB_PER_SB + g
                for ic in range(IT // 128):
                    lhsT = pt[:, g, 128 * ic:128 * (ic + 1)]
                    for ab in range(2):
                        rid = ic * 2 + ab
                        rhs = (vt[:, jblk, 0:129] if ab == 0
                               else vt[:, jblk, 129:257])
                        nc.tensor.matmul(
                            av_region(av_t, rid), lhsT=lhsT, rhs=rhs,
                            # start=True clears the has_written bits of the
                            # whole bank: only the first region per bank may
                            # set it (rids 0,3,6 are first in banks 0,1,2);
                            # the other regions' first writes overwrite via
                            # unset bits.
                            start=(first and g == 0 and rid in (0, 3, 6)),
                            stop=(last and g == JB_PER_SB - 1))

        avc = {}

        def stage_region(rid):
            bank, slot = rid // 3, rid % 3
            w = 129 if rid % 2 == 0 else 128
            off = AV_REGION_STRIDE * slot
            return av_stage[:, bank, off:off + w]

        def epi_stage(av_t):
            for bk in range(3):
                nc.vector.tensor_copy(av_stage[:, bk, :], av_t[bk])

        def epi_chunk_a(it, ic):
            a = stage_region(ic * 2)
            b = stage_region(ic * 2 + 1)
            rden = sm.tile([128, 1], FP32, tag="rden", name=f"rden{it}_{ic}")
            nc.vector.reciprocal(rden, a[:, 128:129])
            avn = sm.tile([128, C], FP16, tag="avn", name=f"avn{it}_{ic}")
            nc.vector.tensor_scalar_mul(avn[:, 0:128], in0=a[:, 0:128],
                                        scalar1=rden)
            nc.vector.tensor_scalar_mul(avn[:, 128:256], in0=b, scalar1=rden)
            tp = epi.tile([128, C], FP16, tag="epi", name=f"tp{it}_{ic}")
            nc.tensor.transpose(tp[:, 0:128], avn[:, 0:128], ident)
            nc.tensor.transpose(tp[:, 128:256], avn[:, 128:256], ident)
            for cc in range(2):
                nc.vector.tensor_copy(
                    avc[(it, cc)][:, 128 * ic:128 * (ic + 1)],
                    tp[:, 128 * cc:128 * (cc + 1)])

        def epi_chunk_b(it, ob, ih):
            isl = slice(IT * it + 256 * ih, IT * it + 256 * (ih + 1))
            pj = epi.tile([128, 256], FP32, tag="epi", name=f"pj{it}_{ob}_{ih}")
            for cc in range(2):
                nc.tensor.matmul(
                    pj, lhsT=wp_sb[cc][:, 128 * ob:128 * (ob + 1)],
                    rhs=avc[(it, cc)][:, 256 * ih:256 * (ih + 1)],
                    start=(cc == 0), stop=(cc == 1))
            nc.vector.bn_stats(stats_sb[:, ob, it, ih], pj)
            nc.vector.tensor_copy(proj_sb[ob][:, isl], pj)
            if use_collective:
                nc.sync.dma_start(ccin_dr[:, ob, it, ih],
                                  stats_sb[:, ob, it, ih])

        pending = []
        pts = {0: emit_qk(0), 1: emit_qk(1)}
        av_t = None
        for s in range(N_SB):
            it, sbj = s // SB_PER_IT, s % SB_PER_IT
            if pending:
                pending.pop(0)()
            if sbj == 0:
                av_t = [avp.tile([128, 512], FP32, tag="av",
                                 name=f"av{it}_{bk}") for bk in range(3)]
            # QK(s+2) is emitted before AV(s): both wait on exp(s) (PSUM
            # buffer WAR / pt data), and the PE must not sit behind AV in
            # the queue when exp(s+1) depends on QK(s+1)
            if s + 2 < N_SB:
                pts[s + 2] = emit_qk(s + 2)
            emit_av(s, pts.pop(s), av_t)
            if use_collective and s == 40:
                # CC-path keepalive: a tiny AllGather mid-loop keeps the
                # collective machinery warm for the real stats exchange
                ka_in = dramp.tile([128, 4], FP32, tag="ka_i", name="ka_in")
                ka_out = dramp.tile([2, 128, 4], FP32, tag="ka_o",
                                    name="ka_out")
                nc.sync.dma_start(ka_in, warm_sb)
                nc.gpsimd.collective_compute(
                    "AllGather", ALU.bypass, replica_groups=REPLICA_GROUPS,
                    ins=[ka_in.opt()], outs=[ka_out.opt()])
            if sbj == SB_PER_IT - 1:
                for cc in range(2):
                    avc[(it, cc)] = avcp.tile([128, IT], FP16, tag=f"avc{cc}",
                                              name=f"avc{it}_{cc}")
                # one fast PSUM->SBUF drain frees the av banks for the next
                # i-tile; the chunk_a/b epilogue then runs off av_stage,
                # spread one unit per superburst
                epi_stage(av_t)
                pending.extend([
                    (lambda it=it: epi_chunk_a(it, 0)),
                    (lambda it=it: epi_chunk_a(it, 1)),
                    (lambda it=it: epi_chunk_b(it, 0, 0)),
                    (lambda it=it: epi_chunk_b(it, 1, 0)),
                    (lambda it=it: epi_chunk_a(it, 2)),
                    (lambda it=it: epi_chunk_a(it, 3)),
                    (lambda it=it: epi_chunk_b(it, 0, 1)),
                    (lambda it=it: epi_chunk_b(it, 1, 1)),
                ])
        while pending:
            pending.pop(0)()

        # ---- cross-core InstanceNorm stats -----------------------------
        mv2 = persist.tile([128, 2, 2], FP32, tag="mv2", name="mv2")
        if use_collective:
            nc.gpsimd.collective_compute(
                "AllGather", ALU.bypass, replica_groups=REPLICA_GROUPS,
                ins=[ccin_dr.opt()], outs=[ccout_dr.opt()])
            cc16 = persist.tile([128, 2, 2, N_ITILES, 2, 6], FP32, tag="cc16",
                                name="cc16")
            for r in range(2):
                nc.sync.dma_start(cc16[:, :, r], ccout_dr[r])
            for ob in range(2):
                nc.vector.bn_aggr(
                    out=mv2[:, ob],
                    in_=cc16[:, ob].rearrange("p r i h s -> p (r i h) s"))
        else:
            for ob in range(2):
                nc.vector.bn_aggr(
                    out=mv2[:, ob],
                    in_=stats_sb[:, ob].rearrange("p i h s -> p (i h) s"))

        # rstd = 1/sqrt(var + eps); negmr = -mean * rstd
        rstd2 = persist.tile([128, 2], FP32, tag="rstd2", name="rstd2")
        nc.scalar.activation(out=rstd2, in_=mv2[:, :, 1], func=AF.Sqrt,
                             bias=eps_sb, scale=1.0)
        nc.vector.reciprocal(rstd2, rstd2)
        negmr = persist.tile([128, 2], FP32, tag="negmr", name="negmr")
        nc.vector.tensor_mul(negmr, mv2[:, :, 0], rstd2)
        nc.vector.tensor_scalar_mul(negmr, in0=negmr, scalar1=-1.0)

        # ---- final norm + residual + store -----------------------------
        stq = [nc.sync, nc.gpsimd]
        for ob in range(2):
            rstd = rstd2[:, ob:ob + 1]
            nmr = negmr[:, ob:ob + 1]
            for ch in range(4):
                sl = slice(512 * ch, 512 * (ch + 1))
                nc.scalar.activation(
                    out=proj_sb[ob][:, sl], in_=proj_sb[ob][:, sl],
                    func=AF.Identity, bias=nmr, scale=rstd)
                nc.vector.tensor_add(proj_sb[ob][:, sl], proj_sb[ob][:, sl],
                                     x1_sb[ob][:, sl])
                stq[(ob * 4 + ch) % 2].dma_start(
                    out_d[128 * ob:128 * (ob + 1), sl], proj_sb[ob][:, sl])


_nc_cache = None


def _get_nc():
    global _nc_cache
    if _nc_cache is None:
        import os
        _nc_cache = build_nc(
            use_collective=not os.environ.get("XATTN_NO_COLLECTIVE"))
    return _nc_cache


def make_in_maps(x1, x2, wq, wk, wv, wp):
    x1f = np.asarray(x1, np.float32).reshape(B, C, N).astype(np.float16)
    x2f = np.asarray(x2, np.float32).reshape(B, C, N).astype(np.float16)
    wvT = np.asarray(wv, np.float32).T
    wkT = np.tile(np.asarray(wk, np.float32).T, (1, 4))
    wqT = np.tile(np.asarray(wq, np.float32).T, (1, 4))
    wpT = np.asarray(wp, np.float32).T
    wcat = np.ascontiguousarray(
        np.concatenate([wvT, wkT, wqT, wpT], axis=1).astype(np.float16))
    in_maps = []
    for core in range(N_CORES):
        b, h = core // 2, core % 2
        in_maps.append({
            "x1h": np.ascontiguousarray(x1f[b, :, h * NH:(h + 1) * NH]),
            "x2b": np.ascontiguousarray(x2f[b]),
            "wcat": wcat,
        })
    return in_maps


def assemble_out(results):
    out = np.empty((B, C, N), np.float32)
    for core in range(N_CORES):
        b, h = core // 2, core % 2
        out[b, :, h * NH:(h + 1) * NH] = results[core]["out"]
    return out.reshape(B, C, 16, 16, 16)


def kernel(**inputs):
    global LAST_RESULTS
    in_maps = make_in_maps(inputs["x1"], inputs["x2"], inputs["wq"],
                           inputs["wk"], inputs["wv"], inputs["wp"])
    res = run_bass_kernel_spmd(_get_nc(), in_maps, core_ids=list(range(N_CORES)))
    LAST_RESULTS = res
    return assemble_out(res.results)


# revision 33
# speedup vs baseline: 1.5277x; 1.1827x over previous
"""Trainium2 Bass kernel for nn_CrossAttention (single-head NxN attention + proj + InstanceNorm + residual).

Sharding: 8 cores = (batch b in 0..3) x (query-half h in 0..1).
Each core computes its half of the query tokens for one batch; the
InstanceNorm statistics (over the full 4096 tokens) are combined across
the core pair with a tiny AllGather (a dummy warmup AllGather at kernel
start absorbs the ~45us first-collective cost).

v3 structure:
 - PE clock warmup: dummy matmuls during the input-DMA prologue keep the
   PE HAM activity monitor busy so the first real matmuls run at 2.4 GHz.
 - Coarse input DMA: weights are host-concatenated into one tensor and
   the whole input set moves in 8 large descriptors (descriptor issue
   costs ~0.6us each and rings only keep ~2 in flight).
 - i-tiles of 512 queries: QK runs as 4 concurrent 32-row-group matmuls
   (one per j-block) with 512-wide moving operands -- ~3x fewer PE
   cycles than 256-wide 2-way-concurrent tiles.  One exp() activation
   per superburst covers [128, 2048] (less ACT overhead).
 - PSUM: qk [128,4,512] (4 banks) + 3 av accumulator banks holding 8
   packed regions (per 128-query chunk: [c0..127+denom] and [c128..255])
   + 1 bank for transpose/proj/v-prologue tiles.
 - Raw per-i-tile bn_stats are exchanged in the AllGather, staged into
   the collective input DRAM as produced; the pair combination is two
   bn_aggr calls.

Precision: fp16 matmul operands everywhere, fp32 PSUM accumulation; the
qk*Cr^-0.5 scale is folded into the exp() activation scale.

Self-contained: hardcodes shapes B=4, C=256, D=H=W=16 (N=4096), Cr=32.
"""

import numpy as np

import concourse.bass as bass
import concourse.mybir as mybir
import concourse.tile as tile
from concourse import bacc
from concourse.bass_utils import run_bass_kernel_spmd
from concourse.masks import make_identity

B, C, N, Cr = 4, 256, 4096, 32
NH = N // 2  # query tokens per core
EPS = 1e-5
SCALE = float(Cr) ** -0.5
FP32 = mybir.dt.float32
FP16 = mybir.dt.float16

N_CORES = 8
REPLICA_GROUPS = [[0, 1], [2, 3], [4, 5], [6, 7]]

IT = 512                   # i-tile width (query columns per superburst)
N_ITILES = NH // IT        # 4
JBLK = 128                 # j-block (rows per QK matmul output)
N_JBLK = N // JBLK         # 32
JB_PER_SB = 2              # j-blocks per superburst (2-way row-tiled QK)
SB_PER_IT = N_JBLK // JB_PER_SB  # 16
N_SB = N_ITILES * SB_PER_IT      # 64

# av accumulator packing: 8 regions (4 query-chunks x {A: c0-127+denom,
# B: c128-255}) packed 3 per PSUM bank at 136-col stride
AV_REGION_STRIDE = 136

N_WARM_MM = 8              # dummy matmuls to warm the PE clock gate

AF = mybir.ActivationFunctionType
ALU = mybir.AluOpType

LAST_RESULTS = None  # BassKernelResults of the most recent run (for test harness)


def build_nc(use_collective=True):
    nc = bacc.Bacc("TRN2", num_devices=N_CORES, name="xattn",
                   target_bir_lowering=False)

    x1h_d = nc.dram_tensor("x1h", [C, NH], FP16, kind="ExternalInput").ap()
    x2b_d = nc.dram_tensor("x2b", [C, N], FP16, kind="ExternalInput").ap()
    # wv[256] | wk[128] | wq[128] | wp[256] concatenated along the free dim
    wcat_d = nc.dram_tensor("wcat", [C, 768], FP16, kind="ExternalInput").ap()
    out_d = nc.dram_tensor("out", [C, NH], FP32, kind="ExternalOutput").ap()

    with tile.TileContext(nc) as tc:
        build_body(tc, x1h_d, x2b_d, wcat_d, out_d, use_collective)
    nc.compile()
    return nc


def build_body(tc, x1h_d, x2b_d, wcat_d, out_d, use_collective=True):
    nc = tc.nc
    from contextlib import ExitStack

    with ExitStack() as ctx:
        persist = ctx.enter_context(tc.tile_pool(name="persist", bufs=1))
        sm = ctx.enter_context(tc.tile_pool(name="sm", bufs=4))
        avcp = ctx.enter_context(tc.tile_pool(name="avcp", bufs=2))
        ptp = ctx.enter_context(tc.tile_pool(name="ptp", bufs=3))
        qkp = ctx.enter_context(tc.tile_pool(name="qkp", bufs=2, space="PSUM"))
        avp = ctx.enter_context(tc.tile_pool(name="avp", bufs=3, space="PSUM"))
        # one bank shared by the prologue vp tiles and the epilogue tp/pj
        # tiles (PSUM pool slots are bank-rounded, so bufs=1)
        epi = ctx.enter_context(tc.tile_pool(name="epi", bufs=1, space="PSUM"))
        dramp = ctx.enter_context(tc.tile_pool(name="dramp", bufs=1, space="DRAM"))

        # ---- PE clock warmup: dummy matmuls on a memset tile ------------
        warm_mm = persist.tile([128, 512], FP16, tag="warm_mm", name="warm_mm")
        nc.vector.memset(warm_mm, 0.0)
        warm_ps = qkp.tile([128, 512], FP32, tag="qk", name="warm_ps")
        for w in range(N_WARM_MM):
            nc.tensor.matmul(warm_ps, lhsT=warm_mm[:, 0:128], rhs=warm_mm,
                             start=True, stop=True)

        # ---- warmup collective ------------------------------------------
        if use_collective:
            warm_sb = persist.tile([128, 4], FP32, tag="warm", name="warm_sb")
            nc.vector.memset(warm_sb, 0.0)
            warm_in = dramp.tile([128, 4], FP32, tag="warm_i", name="warm_in")
            warm_out = dramp.tile([2, 128, 4], FP32, tag="warm_o", name="warm_out")
            nc.sync.dma_start(warm_in, warm_sb)
            nc.gpsimd.collective_compute(
                "AllGather", ALU.bypass, replica_groups=REPLICA_GROUPS,
                ins=[warm_in.opt()], outs=[warm_out.opt()])

        # ---- constants -------------------------------------------------
        eps_sb = persist.tile([128, 1], FP32, tag="eps", name="eps_sb")
        nc.vector.memset(eps_sb, EPS)
        ident = persist.tile([128, 128], FP16, tag="ident", name="ident")
        make_identity(nc, ident)

        # ---- input DMAs (large descriptors, weights first) -------------
        wcat_sb = [persist.tile([128, 768], FP16, tag=f"wcat{cc}",
                                name=f"wcat_sb{cc}") for cc in range(2)]
        wv_sb = [wcat_sb[cc][:, 0:256] for cc in range(2)]
        wk_sb = [wcat_sb[cc][:, 256:384] for cc in range(2)]
        wq_sb = [wcat_sb[cc][:, 384:512] for cc in range(2)]
        wp_sb = [wcat_sb[cc][:, 512:768] for cc in range(2)]
        x2_sb = [persist.tile([128, N], FP16, tag=f"x2_{cc}", name=f"x2_sb{cc}")
                 for cc in range(2)]
        x1_sb = [persist.tile([128, NH], FP16, tag=f"x1_{cc}", name=f"x1_sb{cc}")
                 for cc in range(2)]
        for cc in range(2):
            sl = slice(128 * cc, 128 * (cc + 1))
            nc.scalar.dma_start(wcat_sb[cc], wcat_d[sl, :])
        x2q = [nc.sync, nc.gpsimd]
        for ch in range(2):
            slh = slice(2048 * ch, 2048 * (ch + 1))
            for cc in range(2):
                x2q[cc].dma_start(x2_sb[cc][:, slh],
                                  x2b_d[128 * cc:128 * (cc + 1), slh])
        for cc in range(2):
            nc.scalar.dma_start(x1_sb[cc], x1h_d[128 * cc:128 * (cc + 1), :])

        # ---- prologue: vt / k_rep / q_rep ------------------------------
        # vt[j, :] = [v(c0..127) | ones | v(c128..255)] per j-block; the
        # ones column rides the A-half AV matmul as the softmax denominator
        vt = persist.tile([128, N_JBLK, C + 1], FP16, tag="vt", name="vt")
        nc.vector.memset(vt[:, :, 128:129], 1.0)
        k_rep = persist.tile([128, N], FP16, tag="krep", name="k_rep")
        q_rep = persist.tile([128, NH], FP16, tag="qrep", name="q_rep")

        def emit_v(jblk):
            # vp tiles rotate through 4 PSUM slots (epi + the 3 av banks,
            # which are free until the first AV matmul) so the
            # matmul->copy->matmul chain never serializes on one slot
            if jblk % 4 == 0:
                vp = epi.tile([128, C], FP32, tag="epi", name=f"vp{jblk}")
            else:
                vp = avp.tile([128, C], FP32, tag="av", name=f"vp{jblk}")
            for cc in range(2):
                nc.tensor.matmul(
                    vp, lhsT=x2_sb[cc][:, 128 * jblk:128 * (jblk + 1)],
                    rhs=wv_sb[cc], start=(cc == 0), stop=(cc == 1))
            if jblk % 2 == 0:
                nc.vector.tensor_copy(vt[:, jblk, 0:128], vp[:, 0:128])
                nc.scalar.copy(vt[:, jblk, 129:257], vp[:, 128:256])
            else:
                nc.scalar.copy(vt[:, jblk, 0:128], vp[:, 0:128])
                nc.vector.tensor_copy(vt[:, jblk, 129:257], vp[:, 128:256])

        def emit_kq(which, u):
            w_sb, dst = (wk_sb, k_rep) if which == "k" else (wq_sb, q_rep)
            src = x2_sb if which == "k" else x1_sb
            kp = qkp.tile([128, 512], FP32, tag="qk", name=f"{which}p{u}")
            for cc in range(2):
                nc.tensor.matmul(
                    kp, lhsT=w_sb[cc],
                    rhs=src[cc][:, 512 * u:512 * (u + 1)],
                    start=(cc == 0), stop=(cc == 1))
            nc.vector.tensor_copy(dst[:, 512 * u:512 * (u + 1)], kp)

        # k/q/v prologue, interleaved so the vp/kp copy-latency chains hide
        # behind each other's matmuls (x2 chunk 0 covers k0-3/v0-15)
        for u in range(8):
            emit_kq("k", u)
            if u < 4:
                emit_kq("q", u)
            for jblk in range(4 * u, 4 * (u + 1)):
                emit_v(jblk)

        # ---- persistent attention outputs ------------------------------
        proj_sb = [persist.tile([128, NH], FP32, tag=f"proj{ob}",
                                name=f"proj_sb{ob}") for ob in range(2)]
        # [ob, it, ih, 6] layout: (r, it, ih) merges into one bn_aggr axis
        stats_sb = persist.tile([128, 2, N_ITILES, 2, 6], FP32, tag="stats",
                                name="stats_sb")
        # av accumulators are drained to SBUF in one fast copy per i-tile so
        # the next i-tile's AV matmuls only wait ~1us, not the whole epilogue
        av_stage = persist.tile([128, 3, 512], FP32, tag="avst",
                                name="av_stage")
        if use_collective:
            ccin_dr = dramp.tile([128, 2, N_ITILES, 2, 6], FP32, tag="ccin_d",
                                 name="ccin_dr")
            ccout_dr = dramp.tile([2, 128, 2, N_ITILES, 2, 6], FP32,
                                  tag="ccout_d", name="ccout_dr")

        # ---- main loop: 32 superbursts of (4-way QK, one exp, AV) ------
        def emit_qk(s):
            it, sbj = s // SB_PER_IT, s % SB_PER_IT
            isl = slice(IT * it, IT * (it + 1))
            qk = qkp.tile([128, JB_PER_SB, IT], FP32, tag="qk", name=f"qk{s}")
            # alternate row-group pairs per superburst so the next burst's
            # LDWEIGHTS can pull ahead of in-flight matmuls
            gb = (sbj % 2) * 2
            for g in range(JB_PER_SB):
                jblk = sbj * JB_PER_SB + g
                rg = 32 * (gb + g)
                nc.tensor.matmul(
                    qk[:, g, :],
                    lhsT=k_rep[rg:rg + 32,
                               JBLK * jblk:JBLK * (jblk + 1)],
                    rhs=q_rep[rg:rg + 32, isl],
                    start=True, stop=True, tile_position=(rg, 0))
            pt = ptp.tile([128, JB_PER_SB, IT], FP16, tag="pt", name=f"pt{s}")
            nc.scalar.activation(out=pt, in_=qk, func=AF.Exp, scale=SCALE)
            return pt

        def av_region(av_t, rid):
            bank, slot = rid // 3, rid % 3
            w = 129 if rid % 2 == 0 else 128
            off = AV_REGION_STRIDE * slot
            return av_t[bank][:, off:off + w]

        def emit_av(s, pt, av_t):
            it, sbj = s // SB_PER_IT, s % SB_PER_IT
            first = sbj == 0
            last = sbj == SB_PER_IT - 1
            for g in range(JB_PER_SB):
                jblk = sbj * JB_PER_SB + g
                for ic in range(IT // 128):
                    lhsT = pt[:, g, 128 * ic:128 * (ic + 1)]
                    for ab in range(2):
                        rid = ic * 2 + ab
                        rhs = (vt[:, jblk, 0:129] if ab == 0
                               else vt[:, jblk, 129:257])
                        nc.tensor.matmul(
                            av_region(av_t, rid), lhsT=lhsT, rhs=rhs,
                            # start=True clears the has_written bits of the
                            # whole bank: only the first region per bank may
                            # set it (rids 0,3,6 are first in banks 0,1,2);
                            # the other regions' first writes overwrite via
                            # unset bits.
                            start=(first and g == 0 and rid in (0, 3, 6)),
                            stop=(last and g == JB_PER_SB - 1))

        avc = {}

        def stage_region(rid):
            bank, slot = rid // 3, rid % 3
            w = 129 if rid % 2 == 0 else 128
            off = AV_REGION_STRIDE * slot
            return av_stage[:, bank, off:off + w]

        def epi_stage(av_t):
            for bk in range(3):
                nc.vector.tensor_copy(av_stage[:, bk, :], av_t[bk])

        def epi_chunk_a(it, ic):
            a = stage_region(ic * 2)
            b = stage_region(ic * 2 + 1)
            rden = sm.tile([128, 1], FP32, tag="rden", name=f"rden{it}_{ic}")
            nc.vector.reciprocal(rden, a[:, 128:129])
            avn = sm.tile([128, C], FP16, tag="avn", name=f"avn{it}_{ic}")
            nc.vector.tensor_scalar_mul(avn[:, 0:128], in0=a[:, 0:128],
                                        scalar1=rden)
            nc.vector.tensor_scalar_mul(avn[:, 128:256], in0=b, scalar1=rden)
            tp = epi.tile([128, C], FP16, tag="epi", name=f"tp{it}_{ic}")
            nc.tensor.transpose(tp[:, 0:128], avn[:, 0:128], ident)
            nc.tensor.transpose(tp[:, 128:256], avn[:, 128:256], ident)
            for cc in range(2):
                nc.vector.tensor_copy(
                    avc[(it, cc)][:, 128 * ic:128 * (ic + 1)],
                    tp[:, 128 * cc:128 * (cc + 1)])

        def epi_chunk_b(it, ob, ih):
            isl = slice(IT * it + 256 * ih, IT * it + 256 * (ih + 1))
            pj = epi.tile([128, 256], FP32, tag="epi", name=f"pj{it}_{ob}_{ih}")
            for cc in range(2):
                nc.tensor.matmul(
                    pj, lhsT=wp_sb[cc][:, 128 * ob:128 * (ob + 1)],
                    rhs=avc[(it, cc)][:, 256 * ih:256 * (ih + 1)],
                    start=(cc == 0), stop=(cc == 1))
            nc.vector.bn_stats(stats_sb[:, ob, it, ih], pj)
            nc.vector.tensor_copy(proj_sb[ob][:, isl], pj)
            if use_collective:
                nc.sync.dma_start(ccin_dr[:, ob, it, ih],
                                  stats_sb[:, ob, it, ih])

        pending = []
        pts = {0: emit_qk(0), 1: emit_qk(1)}
        av_t = None
        for s in range(N_SB):
            it, sbj = s // SB_PER_IT, s % SB_PER_IT
            if pending:
                pending.pop(0)()
            if sbj == 0:
                av_t = [avp.tile([128, 512], FP32, tag="av",
                                 name=f"av{it}_{bk}") for bk in range(3)]
            # QK(s+2) is emitted before AV(s): both wait on exp(s) (PSUM
            # buffer WAR / pt data), and the PE must not sit behind AV in
            # the queue when exp(s+1) depends on QK(s+1)
            if s + 2 < N_SB:
                pts[s + 2] = emit_qk(s + 2)
            emit_av(s, pts.pop(s), av_t)
            if sbj == SB_PER_IT - 1:
                for cc in range(2):
                    avc[(it, cc)] = avcp.tile([128, IT], FP16, tag=f"avc{cc}",
                                              name=f"avc{it}_{cc}")
                # one fast PSUM->SBUF drain frees the av banks for the next
                # i-tile; the chunk_a/b epilogue then runs off av_stage,
                # spread one unit per superburst
                epi_stage(av_t)
                pending.extend([
                    (lambda it=it: epi_chunk_a(it, 0)),
                    (lambda it=it: epi_chunk_a(it, 1)),
                    (lambda it=it: epi_chunk_b(it, 0, 0)),
                    (lambda it=it: epi_chunk_b(it, 1, 0)),
                    (lambda it=it: epi_chunk_a(it, 2)),
                    (lambda it=it: epi_chunk_a(it, 3)),
                    (lambda it=it: epi_chunk_b(it, 0, 1)),
                    (lambda it=it: epi_chunk_b(it, 1, 1)),
                ])
        while pending:
            pending.pop(0)()

        # ---- cross-core InstanceNorm stats -----------------------------
        mv2 = persist.tile([128, 2, 2], FP32, tag="mv2", name="mv2")
        if use_collective:
            # CC-path keepalives: tiny AllGathers whose inputs depend on
            # i-tile 2 / i-tile 3 stats, so they fire late in the loop and
            # keep the collective machinery warm for the real exchange
            for ki, (kit, kih) in enumerate([(2, 0), (3, 0)]):
                ka_in = dramp.tile([128, 6], FP32, tag=f"ka_i{ki}",
                                   name=f"ka_in{ki}")
                ka_out = dramp.tile([2, 128, 6], FP32, tag=f"ka_o{ki}",
                                    name=f"ka_out{ki}")
                nc.sync.dma_start(ka_in, stats_sb[:, 0, kit, kih])
                nc.gpsimd.collective_compute(
                    "AllGather", ALU.bypass, replica_groups=REPLICA_GROUPS,
                    ins=[ka_in.opt()], outs=[ka_out.opt()])
            nc.gpsimd.collective_compute(
                "AllGather", ALU.bypass, replica_groups=REPLICA_GROUPS,
                ins=[ccin_dr.opt()], outs=[ccout_dr.opt()])
            cc16 = persist.tile([128, 2, 2, N_ITILES, 2, 6], FP32, tag="cc16",
                                name="cc16")
            for r in range(2):
                nc.sync.dma_start(cc16[:, :, r], ccout_dr[r])
            for ob in range(2):
                nc.vector.bn_aggr(
                    out=mv2[:, ob],
                    in_=cc16[:, ob].rearrange("p r i h s -> p (r i h) s"))
        else:
            for ob in range(2):
                nc.vector.bn_aggr(
                    out=mv2[:, ob],
                    in_=stats_sb[:, ob].rearrange("p i h s -> p (i h) s"))

        # rstd = 1/sqrt(var + eps); negmr = -mean * rstd
        rstd2 = persist.tile([128, 2], FP32, tag="rstd2", name="rstd2")
        nc.scalar.activation(out=rstd2, in_=mv2[:, :, 1], func=AF.Sqrt,
                             bias=eps_sb, scale=1.0)
        nc.vector.reciprocal(rstd2, rstd2)
        negmr = persist.tile([128, 2], FP32, tag="negmr", name="negmr")
        nc.vector.tensor_mul(negmr, mv2[:, :, 0], rstd2)
        nc.vector.tensor_scalar_mul(negmr, in0=negmr, scalar1=-1.0)

        # ---- final norm + residual + store -----------------------------
        stq = [nc.sync, nc.gpsimd]
        for ob in range(2):
            rstd = rstd2[:, ob:ob + 1]
            nmr = negmr[:, ob:ob + 1]
            for ch in range(4):
                sl = slice(512 * ch, 512 * (ch + 1))
                nc.scalar.activation(
                    out=proj_sb[ob][:, sl], in_=proj_sb[ob][:, sl],
                    func=AF.Identity, bias=nmr, scale=rstd)
                nc.vector.tensor_add(proj_sb[ob][:, sl], proj_sb[ob][:, sl],
                                     x1_sb[ob][:, sl])
                stq[(ob * 4 + ch) % 2].dma_start(
                    out_d[128 * ob:128 * (ob + 1), sl], proj_sb[ob][:, sl])


_nc_cache = None


def _get_nc():
    global _nc_cache
    if _nc_cache is None:
        import os
        _nc_cache = build_nc(
            use_collective=not os.environ.get("XATTN_NO_COLLECTIVE"))
    return _nc_cache


def make_in_maps(x1, x2, wq, wk, wv, wp):
    x1f = np.asarray(x1, np.float32).reshape(B, C, N).astype(np.float16)
    x2f = np.asarray(x2, np.float32).reshape(B, C, N).astype(np.float16)
    wvT = np.asarray(wv, np.float32).T
    wkT = np.tile(np.asarray(wk, np.float32).T, (1, 4))
    wqT = np.tile(np.asarray(wq, np.float32).T, (1, 4))
    wpT = np.asarray(wp, np.float32).T
    wcat = np.ascontiguousarray(
        np.concatenate([wvT, wkT, wqT, wpT], axis=1).astype(np.float16))
    in_maps = []
    for core in range(N_CORES):
        b, h = core // 2, core % 2
        in_maps.append({
            "x1h": np.ascontiguousarray(x1f[b, :, h * NH:(h + 1) * NH]),
            "x2b": np.ascontiguousarray(x2f[b]),
            "wcat": wcat,
        })
    return in_maps


def assemble_out(results):
    out = np.empty((B, C, N), np.float32)
    for core in range(N_CORES):
        b, h = core // 2, core % 2
        out[b, :, h * NH:(h + 1) * NH] = results[core]["out"]
    return out.reshape(B, C, 16, 16, 16)


def kernel(**inputs):
    global LAST_RESULTS
    in_maps = make_in_maps(inputs["x1"], inputs["x2"], inputs["wq"],
                           inputs["wk"], inputs["wv"], inputs["wp"])
    res = run_bass_kernel_spmd(_get_nc(), in_maps, core_ids=list(range(N_CORES)))
    LAST_RESULTS = res
    return assemble_out(res.results)
